# revision 7
# baseline (speedup 1.0000x reference)
"""Fused multi-head attention (B=2, T=2048, D=2048, H=16) on 8 trn2 NeuronCores.

Sharding: core c handles batch b=c//4 and heads [4g, 4g+4), g=c%4 (tensor
parallel over heads x data parallel over batch). Each core computes its
4 heads' contribution to out[b] = attn(x[b]) @ Wo^T; the host sums the 4
partials per batch.

v2: single fused loop over 512-token chunks (causality: chunk ic's attention
only needs K/V from chunks <= ic), all-bf16 matmul operands (f32 PSUM),
V projected directly into [token, feature] layout (no PE transposes),
diagonal attention blocks computed at partial width with one shared
128x128 triangular mask constant.

Per chunk ic (tokens [512*ic, 512*ic+512)):
  P1  qT[m][:, chunk] = (Wq_s/sqrt(dh) @ x^T)   per m (4 feature tiles)
      kT[m][:, chunk] =  Wk_s @ x^T
      v[4ic+jl]       =  x-block^T-stationary @ Wv  -> [tok, feat]
  P2  per head h: for each surviving key block jt (descending col offset):
        S^T = kT-block^T-contract @ qT[:, off:]  (PSUM)
        pt  = exp(S^T)  (ACT, bf16)   [triangular sub-block *= tri]
        ctx^T[:, off:] += v-block^T @ pt ; l[off:] += 1^T @ pt
      ctx[h][:, chunk] = cps * broadcast(1/l)
  P3  out[t-block, :] = sum_e ctx^T[e, t-block] @ Wo -> DRAM (f32)
"""

import numpy as np
import ml_dtypes

import concourse.bass as bass
import concourse.mybir as mybir
import concourse.tile as tile
from concourse import bacc
from concourse.bass_utils import run_bass_kernel_spmd

F32 = mybir.dt.float32
F32R = mybir.dt.float32r
BF16 = mybir.dt.bfloat16
EXP = mybir.ActivationFunctionType.Exp
BF = ml_dtypes.bfloat16

B, T, D, H = 2, 2048, 2048, 16
DH = D // H          # 128
E = 512              # features per core (4 heads)
HPC = 4              # heads per core
NT = T // 128        # 16 token tiles
ND = D // 128        # 16 model-dim tiles
NE = E // 128        # 4 e-tiles per core
NI = T // 512        # 4 token chunks
NJ = NT              # 16 key tiles

_NC_CACHE = {}


def _build(blocks_key):
    # blocks_key: tuple over ic of tuple of (jt, off, mixed_tuple) where
    # mixed_tuple is ((c, pat_idx), ...) for 128-col sub-blocks needing an
    # elementwise mask multiply; n_pat = number of distinct mask patterns.
    blocks_per_ic, n_pat = blocks_key
    nc = bacc.Bacc(None, target_bir_lowering=False, debug=False)
    xt = nc.declare_dram_parameter("xt", [D, T], BF16, isOutput=False)
    wq = nc.declare_dram_parameter("wq", [D, E], BF16, isOutput=False)
    wk = nc.declare_dram_parameter("wk", [D, E], BF16, isOutput=False)
    wv = nc.declare_dram_parameter("wv", [D, E], BF16, isOutput=False)
    wo = nc.declare_dram_parameter("wo", [E, D], BF16, isOutput=False)
    em = nc.declare_dram_parameter("em", [128, n_pat, 128], BF16, isOutput=False)
    onk = nc.declare_dram_parameter("onk", [128, 1], BF16, isOutput=False)
    out = nc.declare_dram_parameter("out", [T, D], F32, isOutput=True)

    with tile.TileContext(nc) as tc:
        # ---- long-lived residents ---------------------------------------
        p_res = tc.alloc_tile_pool(name="res", bufs=1)
        qT = [p_res.tile([128, T], BF16, name=f"qT{m}") for m in range(NE)]
        kT = [p_res.tile([128, T], BF16, name=f"kT{m}") for m in range(NE)]
        ctx = [p_res.tile([128, T], BF16, name=f"ctx{m}") for m in range(NE)]
        v_sb = p_res.tile([128, NT, E], BF16)
        wq_sb = p_res.tile([128, ND, E], BF16)
        wk_sb = p_res.tile([128, ND, E], BF16)
        wv_sb = p_res.tile([128, ND, E], BF16)
        wo_sb = p_res.tile([128, NE, D], BF16)
        em_sb = p_res.tile([128, n_pat, 128], BF16)
        onk_sb = p_res.tile([128, 1], BF16)

        # ---- working pools ----------------------------------------------
        p_x = tc.alloc_tile_pool(name="px", bufs=2)
        p_pt = tc.alloc_tile_pool(name="ppt", bufs=4)
        p_ot = tc.alloc_tile_pool(name="pot", bufs=3)
        p_bs = tc.alloc_tile_pool(name="pbs", bufs=2)
        p_rr = tc.alloc_tile_pool(name="prr", bufs=2)
        ps_big = tc.alloc_tile_pool(name="psbig", bufs=3, space="PSUM")
        ps_cps = tc.alloc_tile_pool(name="pscps", bufs=2, space="PSUM")
        ps_sm = tc.alloc_tile_pool(name="pssm", bufs=1, space="PSUM")

        # DMA emission in first-use order: chunk-0 x interleaved with q/k
        # weights (P1 needs both immediately), then v weights, mask consts
        # (P2), and wo last (first P3 is ~80us in).
        xcs = {}
        xcs[0] = p_x.tile([128, ND, 512], BF16, name="xc", bufs=2)
        for dt in range(ND):
            nc.sync.dma_start(
                out=xcs[0][:, dt, :], in_=xt.ap()[dt * 128:(dt + 1) * 128, 0:512])
            nc.sync.dma_start(out=wq_sb[:, dt, :], in_=wq.ap()[dt * 128:(dt + 1) * 128, :])
            nc.sync.dma_start(out=wk_sb[:, dt, :], in_=wk.ap()[dt * 128:(dt + 1) * 128, :])
        for dt in range(ND):
            nc.sync.dma_start(out=wv_sb[:, dt, :], in_=wv.ap()[dt * 128:(dt + 1) * 128, :])
        nc.sync.dma_start(out=em_sb[:, :, :], in_=em.ap())
        nc.sync.dma_start(out=onk_sb, in_=onk.ap())
        for et in range(NE):
            nc.sync.dma_start(out=wo_sb[:, et, :], in_=wo.ap()[et * 128:(et + 1) * 128, :])

        for ic in range(NI):
            csl = slice(ic * 512, (ic + 1) * 512)
            scope = nc.named_scope(f"chunk{ic}")
            scope.__enter__()

            # ---- P1: projections for this chunk -------------------------
            # prefetch next chunk's x ahead of this chunk's output stores
            if ic + 1 < NI:
                nsl = slice((ic + 1) * 512, (ic + 2) * 512)
                xcs[ic + 1] = p_x.tile([128, ND, 512], BF16, name="xc", bufs=2)
                for dt in range(ND):
                    nc.sync.dma_start(
                        out=xcs[ic + 1][:, dt, :],
                        in_=xt.ap()[dt * 128:(dt + 1) * 128, nsl])
            xc = xcs.pop(ic)
            for m in range(NE):
                msl = slice(m * 128, (m + 1) * 128)
                psq = ps_big.tile([128, 512], F32, name="ps", bufs=3)
                psk = ps_big.tile([128, 512], F32, name="ps", bufs=3)
                if ic == 0 and m == 0:
                    # start of kernel is DMA-bound: q sweep first (needs only
                    # xc+wq), k sweep second while wk still streams in
                    for dt in range(ND):
                        nc.tensor.matmul(psq, wq_sb[:, dt, msl], xc[:, dt, :],
                                         start=dt == 0, stop=dt == ND - 1)
                    for dt in range(ND):
                        nc.tensor.matmul(psk, wk_sb[:, dt, msl], xc[:, dt, :],
                                         start=dt == 0, stop=dt == ND - 1)
                else:
                    for dt in range(ND):
                        st, sp = dt == 0, dt == ND - 1
                        nc.tensor.matmul(psq, wq_sb[:, dt, msl], xc[:, dt, :],
                                         start=st, stop=sp)
                        nc.tensor.matmul(psk, wk_sb[:, dt, msl], xc[:, dt, :],
                                         start=st, stop=sp)
                nc.scalar.copy(qT[m][:, csl], psq)
                nc.vector.tensor_copy(kT[m][:, csl], psk)
            for jl in range(4):
                jt = ic * 4 + jl
                psv = ps_big.tile([128, 512], F32, name="ps", bufs=3)
                for dt in range(ND):
                    nc.tensor.matmul(
                        psv, xc[:, dt, jl * 128:(jl + 1) * 128], wv_sb[:, dt, :],
                        start=(dt == 0), stop=(dt == ND - 1))
                nc.vector.tensor_copy(v_sb[:, jt, :], psv)

            # ---- P2: attention for this chunk ---------------------------
            blocks = blocks_per_ic[ic]
            nb = len(blocks)
            for h in range(HPC):
                hsl = slice(h * 128, (h + 1) * 128)
                cps = ps_cps.tile([128, 512], F32, name="cps", bufs=3)
                lps = ps_sm.tile([1, 512], F32, name="lps", bufs=2)
                for bi, (jt, off, mixed) in enumerate(blocks):
                    ps_s = ps_big.tile([128, 512], F32, name="ps", bufs=3)
                    nc.tensor.matmul(
                        ps_s[:, off:512], kT[h][:, jt * 128:(jt + 1) * 128],
                        qT[h][:, ic * 512 + off:(ic + 1) * 512],
                        start=True, stop=True)
                    pt = p_pt.tile([128, 512], BF16, name="pt", bufs=4)
                    nc.scalar.activation(pt[:, off:512], ps_s[:, off:512], EXP)
                    for (c, pidx) in mixed:
                        nc.vector.tensor_mul(
                            pt[:, c * 128:(c + 1) * 128],
                            pt[:, c * 128:(c + 1) * 128],
                            em_sb[:, pidx, :])
                    st, sp = bi == 0, bi == nb - 1
                    nc.tensor.matmul(cps[:, off:512], v_sb[:, jt, hsl],
                                     pt[:, off:512], start=st, stop=sp)
                    nc.tensor.matmul(lps[:, off:512], onk_sb, pt[:, off:512],
                                     start=st, stop=sp)
                rr = p_rr.tile([1, 512], F32, name="rr", bufs=2)
                nc.vector.reciprocal_approx_fast(out=rr, in_=lps)
                rrb = p_bs.tile([128, 512], F32, name="rrb", bufs=2)
                nc.gpsimd.partition_broadcast(rrb, rr)
                nc.vector.tensor_mul(ctx[h][:, csl], cps, rrb)

            # ---- P3: output projection for this chunk's tokens ----------
            for tl in range(4):
                tt = ic * 4 + tl
                tsl = slice(tt * 128, (tt + 1) * 128)
                for nch in range(NI):
                    ps_o = ps_big.tile([128, 512], F32, name="ps", bufs=3)
                    for et in range(NE):
                        nc.tensor.matmul(
                            ps_o, ctx[et][:, tsl],
                            wo_sb[:, et, nch * 512:(nch + 1) * 512],
                            start=(et == 0), stop=(et == NE - 1))
                    ot = p_ot.tile([128, 512], F32, name="ot", bufs=3)
                    if (tl + nch) % 4 == 0:
                        nc.scalar.copy(ot, ps_o)
                    else:
                        nc.vector.tensor_copy(ot, ps_o)
                    nc.sync.dma_start(
                        out=out.ap()[tsl, nch * 512:(nch + 1) * 512], in_=ot)
            scope.__exit__(None, None, None)

        for p in (ps_sm, ps_cps, ps_big, p_rr, p_bs, p_ot, p_pt, p_x, p_res):
            p.release()

    nc.compile()
    return nc


def _classify(mask):
    """Per (ic, jt): column offset + mixed 128-col sub-blocks, from exp(mask)^T."""
    emT = np.ascontiguousarray(np.exp(mask).T)  # [key j, query i]
    pats = {}   # pattern bytes -> index
    pat_list = []
    blocks_per_ic = []
    for ic in range(NI):
        blk = []
        for jt in range(NJ):
            sub = emT[jt * 128:(jt + 1) * 128, ic * 512:(ic + 1) * 512]
            # 128-col sub-block classes
            kinds = []
            for c in range(4):
                s = sub[:, c * 128:(c + 1) * 128]
                if not s.any():
                    kinds.append(0)
                elif np.all(s == 1.0):
                    kinds.append(1)
                else:
                    kinds.append(2)
            if all(k == 0 for k in kinds):
                continue
            first = next(i for i, k in enumerate(kinds) if k != 0)
            off = first * 128
            mixed = []
            for c in range(first, 4):
                if kinds[c] != 1:
                    s = np.asarray(sub[:, c * 128:(c + 1) * 128], dtype=np.float32)
                    key = s.tobytes()
                    if key not in pats:
                        pats[key] = len(pat_list)
                        pat_list.append(s)
                    mixed.append((c, pats[key]))
            blk.append((jt, off, tuple(mixed)))
        # descending offset so the last block is full width (clean stop)
        blk.sort(key=lambda b: -b[1])
        assert blk and blk[-1][1] == 0, f"ic {ic}: no full-width block"
        blocks_per_ic.append(tuple(blk))
    em_arr = (np.concatenate(pat_list, axis=1) if pat_list
              else np.zeros((128, 128), dtype=np.float32))
    return tuple(blocks_per_ic), max(1, len(pat_list)), em_arr


def kernel(x, Wq, Wk, Wv, Wo, attn_mask):
    x = np.asarray(x, dtype=np.float32)
    Wq = np.asarray(Wq, dtype=np.float32)
    Wk = np.asarray(Wk, dtype=np.float32)
    Wv = np.asarray(Wv, dtype=np.float32)
    Wo = np.asarray(Wo, dtype=np.float32)
    mask = np.asarray(attn_mask, dtype=np.float32).reshape(T, T)

    blocks_per_ic, n_pat, em_arr = _classify(mask)
    scale = np.float32(1.0 / np.sqrt(DH))

    xT = [np.ascontiguousarray(x[b].T).astype(BF) for b in range(B)]
    em_bf = np.ascontiguousarray(em_arr).astype(BF)

    in_maps = []
    for c in range(8):
        b, g = c // 4, c % 4
        rows = slice(E * g, E * (g + 1))
        in_maps.append({
            "xt": xT[b],
            "wq": np.ascontiguousarray((Wq[rows, :] * scale).T).astype(BF),
            "wk": np.ascontiguousarray(Wk[rows, :].T).astype(BF),
            "wv": np.ascontiguousarray(Wv[rows, :].T).astype(BF),
            "wo": np.ascontiguousarray(Wo[:, rows].T).astype(BF),
            "em": em_bf.reshape(128, n_pat, 128),
            "onk": np.ones((128, 1), dtype=BF),
        })

    global _LAST_IN_MAPS, _LAST_NC
    _LAST_IN_MAPS = in_maps
    key = (blocks_per_ic, n_pat)
    if key not in _NC_CACHE:
        _NC_CACHE[key] = _build(key)
    nc = _NC_CACHE[key]
    _LAST_NC = nc
    res = run_bass_kernel_spmd(nc, in_maps, list(range(8)))
    outs = [np.asarray(r["out"], dtype=np.float32) for r in res.results]
    full = np.stack([
        outs[0] + outs[1] + outs[2] + outs[3],
        outs[4] + outs[5] + outs[6] + outs[7],
    ]).astype(np.float32)
    return full


# revision 8
# speedup vs baseline: 1.0140x; 1.0140x over previous
"""Fused multi-head attention (B=2, T=2048, D=2048, H=16) on 8 trn2 NeuronCores.

Sharding: core c handles batch b=c//4 and heads [4g, 4g+4), g=c%4 (tensor
parallel over heads x data parallel over batch). Each core computes its
4 heads' contribution to out[b] = attn(x[b]) @ Wo^T; the host sums the 4
partials per batch.

v2: single fused loop over 512-token chunks (causality: chunk ic's attention
only needs K/V from chunks <= ic), all-bf16 matmul operands (f32 PSUM),
V projected directly into [token, feature] layout (no PE transposes),
diagonal attention blocks computed at partial width with one shared
128x128 triangular mask constant.

Per chunk ic (tokens [512*ic, 512*ic+512)):
  P1  qT[m][:, chunk] = (Wq_s/sqrt(dh) @ x^T)   per m (4 feature tiles)
      kT[m][:, chunk] =  Wk_s @ x^T
      v[4ic+jl]       =  x-block^T-stationary @ Wv  -> [tok, feat]
  P2  per head h: for each surviving key block jt (descending col offset):
        S^T = kT-block^T-contract @ qT[:, off:]  (PSUM)
        pt  = exp(S^T)  (ACT, bf16)   [triangular sub-block *= tri]
        ctx^T[:, off:] += v-block^T @ pt ; l[off:] += 1^T @ pt
      ctx[h][:, chunk] = cps * broadcast(1/l)
  P3  out[t-block, :] = sum_e ctx^T[e, t-block] @ Wo -> DRAM (f32)
"""

import numpy as np
import ml_dtypes

import concourse.bass as bass
import concourse.mybir as mybir
import concourse.tile as tile
from concourse import bacc
from concourse.bass_utils import run_bass_kernel_spmd

F32 = mybir.dt.float32
F32R = mybir.dt.float32r
BF16 = mybir.dt.bfloat16
EXP = mybir.ActivationFunctionType.Exp
BF = ml_dtypes.bfloat16

B, T, D, H = 2, 2048, 2048, 16
DH = D // H          # 128
E = 512              # features per core (4 heads)
HPC = 4              # heads per core
NT = T // 128        # 16 token tiles
ND = D // 128        # 16 model-dim tiles
NE = E // 128        # 4 e-tiles per core
NI = T // 512        # 4 token chunks
NJ = NT              # 16 key tiles

_NC_CACHE = {}


def _build(blocks_key):
    # blocks_key: tuple over ic of tuple of (jt, off, mixed_tuple) where
    # mixed_tuple is ((c, pat_idx), ...) for 128-col sub-blocks needing an
    # elementwise mask multiply; n_pat = number of distinct mask patterns.
    blocks_per_ic, n_pat = blocks_key
    nc = bacc.Bacc(None, target_bir_lowering=False, debug=False)
    xt = nc.declare_dram_parameter("xt", [D, T], BF16, isOutput=False)
    wq = nc.declare_dram_parameter("wq", [D, E], BF16, isOutput=False)
    wk = nc.declare_dram_parameter("wk", [D, E], BF16, isOutput=False)
    wv = nc.declare_dram_parameter("wv", [D, E], BF16, isOutput=False)
    wo = nc.declare_dram_parameter("wo", [E, D], BF16, isOutput=False)
    em = nc.declare_dram_parameter("em", [128, n_pat, 128], BF16, isOutput=False)
    onk = nc.declare_dram_parameter("onk", [128, 1], BF16, isOutput=False)
    out = nc.declare_dram_parameter("out", [T, D], F32, isOutput=True)

    with tile.TileContext(nc) as tc:
        # ---- long-lived residents ---------------------------------------
        p_res = tc.alloc_tile_pool(name="res", bufs=1)
        qT = [p_res.tile([128, T], BF16, name=f"qT{m}") for m in range(NE)]
        kT = [p_res.tile([128, T], BF16, name=f"kT{m}") for m in range(NE)]
        ctx = [p_res.tile([128, T], BF16, name=f"ctx{m}") for m in range(NE)]
        v_sb = p_res.tile([128, NT, E], BF16)
        wq_sb = p_res.tile([128, ND, E], BF16)
        wk_sb = p_res.tile([128, ND, E], BF16)
        wv_sb = p_res.tile([128, ND, E], BF16)
        wo_sb = p_res.tile([128, NE, D], BF16)
        em_sb = p_res.tile([128, n_pat, 128], BF16)
        onk_sb = p_res.tile([128, 1], BF16)
        z512 = p_res.tile([1, 512], BF16)
        nc.vector.memset(z512, 0.0)

        # ---- working pools ----------------------------------------------
        p_x = tc.alloc_tile_pool(name="px", bufs=2)
        p_pt = tc.alloc_tile_pool(name="ppt", bufs=4)
        p_ot = tc.alloc_tile_pool(name="pot", bufs=3)
        p_bs = tc.alloc_tile_pool(name="pbs", bufs=2)
        p_rr = tc.alloc_tile_pool(name="prr", bufs=2)
        ps_big = tc.alloc_tile_pool(name="psbig", bufs=3, space="PSUM")
        ps_cps = tc.alloc_tile_pool(name="pscps", bufs=2, space="PSUM")
        ps_sm = tc.alloc_tile_pool(name="pssm", bufs=1, space="PSUM")

        # DMA emission in first-use order: chunk-0 x interleaved with q/k
        # weights (P1 needs both immediately), then v weights, mask consts
        # (P2), and wo last (first P3 is ~80us in).
        xcs = {}
        xcs[0] = p_x.tile([128, ND, 512], BF16, name="xc", bufs=2)
        for dt in range(ND):
            nc.sync.dma_start(
                out=xcs[0][:, dt, :], in_=xt.ap()[dt * 128:(dt + 1) * 128, 0:512])
            nc.sync.dma_start(out=wq_sb[:, dt, :], in_=wq.ap()[dt * 128:(dt + 1) * 128, :])
            nc.sync.dma_start(out=wk_sb[:, dt, :], in_=wk.ap()[dt * 128:(dt + 1) * 128, :])
        for dt in range(ND):
            nc.sync.dma_start(out=wv_sb[:, dt, :], in_=wv.ap()[dt * 128:(dt + 1) * 128, :])
        nc.sync.dma_start(out=em_sb[:, :, :], in_=em.ap())
        nc.sync.dma_start(out=onk_sb, in_=onk.ap())
        for et in range(NE):
            nc.sync.dma_start(out=wo_sb[:, et, :], in_=wo.ap()[et * 128:(et + 1) * 128, :])

        for ic in range(NI):
            csl = slice(ic * 512, (ic + 1) * 512)
            scope = nc.named_scope(f"chunk{ic}")
            scope.__enter__()

            # ---- P1: projections for this chunk -------------------------
            # prefetch next chunk's x ahead of this chunk's output stores
            if ic + 1 < NI:
                nsl = slice((ic + 1) * 512, (ic + 2) * 512)
                xcs[ic + 1] = p_x.tile([128, ND, 512], BF16, name="xc", bufs=2)
                for dt in range(ND):
                    nc.sync.dma_start(
                        out=xcs[ic + 1][:, dt, :],
                        in_=xt.ap()[dt * 128:(dt + 1) * 128, nsl])
            xc = xcs.pop(ic)
            for m in range(NE):
                msl = slice(m * 128, (m + 1) * 128)
                psq = ps_big.tile([128, 512], F32, name="ps", bufs=3)
                psk = ps_big.tile([128, 512], F32, name="ps", bufs=3)
                if ic == 0 and m == 0:
                    # start of kernel is DMA-bound: q sweep first (needs only
                    # xc+wq), k sweep second while wk still streams in
                    for dt in range(ND):
                        nc.tensor.matmul(psq, wq_sb[:, dt, msl], xc[:, dt, :],
                                         start=dt == 0, stop=dt == ND - 1)
                    for dt in range(ND):
                        nc.tensor.matmul(psk, wk_sb[:, dt, msl], xc[:, dt, :],
                                         start=dt == 0, stop=dt == ND - 1)
                else:
                    for dt in range(ND):
                        st, sp = dt == 0, dt == ND - 1
                        nc.tensor.matmul(psq, wq_sb[:, dt, msl], xc[:, dt, :],
                                         start=st, stop=sp)
                        nc.tensor.matmul(psk, wk_sb[:, dt, msl], xc[:, dt, :],
                                         start=st, stop=sp)
                nc.scalar.copy(qT[m][:, csl], psq)
                nc.vector.tensor_copy(kT[m][:, csl], psk)
            for jl in range(4):
                jt = ic * 4 + jl
                psv = ps_big.tile([128, 512], F32, name="ps", bufs=3)
                for dt in range(ND):
                    nc.tensor.matmul(
                        psv, xc[:, dt, jl * 128:(jl + 1) * 128], wv_sb[:, dt, :],
                        start=(dt == 0), stop=(dt == ND - 1))
                nc.vector.tensor_copy(v_sb[:, jt, :], psv)

            # ---- P2: attention for this chunk ---------------------------
            # Block-major over heads: the 4 heads' row-sum (l) matmuls are
            # emitted back-to-back with M=1 stationaries in disjoint column
            # groups (tile_position=(0,32h)) of ONE shared PSUM bank, so
            # they execute concurrently in the PE array.  A K=1 zero matmul
            # clears the bank first (start=True clears has_written for the
            # whole bank, so per-head start=True would corrupt the others).
            blocks = blocks_per_ic[ic]
            nb = len(blocks)
            cpss = [ps_cps.tile([128, 512], F32, name="cps", bufs=4)
                    for _ in range(HPC)]
            lps = ps_sm.tile([128, 512], F32, name="lps", bufs=1)
            nc.tensor.matmul(lps, em_sb[0:1, 0, :], z512,
                             start=True, stop=True, skip_group_check=True)
            for bi, (jt, off, mixed) in enumerate(blocks):
                pts = []
                for h in range(HPC):
                    ps_s = ps_big.tile([128, 512], F32, name="ps", bufs=3)
                    nc.tensor.matmul(
                        ps_s[:, off:512], kT[h][:, jt * 128:(jt + 1) * 128],
                        qT[h][:, ic * 512 + off:(ic + 1) * 512],
                        start=True, stop=True)
                    pt = p_pt.tile([128, 512], BF16, name="pt", bufs=6)
                    nc.scalar.activation(pt[:, off:512], ps_s[:, off:512], EXP)
                    for (c, pidx) in mixed:
                        nc.vector.tensor_mul(
                            pt[:, c * 128:(c + 1) * 128],
                            pt[:, c * 128:(c + 1) * 128],
                            em_sb[:, pidx, :])
                    pts.append(pt)
                st, sp = bi == 0, bi == nb - 1
                for h in range(HPC):
                    nc.tensor.matmul(cpss[h][:, off:512],
                                     v_sb[:, jt, h * 128:(h + 1) * 128],
                                     pts[h][:, off:512], start=st, stop=sp)
                for h in range(HPC):
                    nc.tensor.matmul(
                        lps[32 * h:32 * h + 1, off:512], onk_sb,
                        pts[h][:, off:512], start=False, stop=sp,
                        tile_position=(0, 32 * h), skip_group_check=True)
            rr = p_rr.tile([128, 512], F32, name="rr", bufs=2)
            nc.vector.reciprocal_approx_fast(out=rr, in_=lps)
            for h in range(HPC):
                rrb = p_bs.tile([128, 512], F32, name="rrb", bufs=2)
                nc.gpsimd.partition_broadcast(rrb, rr[32 * h:32 * h + 1, :])
                nc.vector.tensor_mul(ctx[h][:, csl], cpss[h], rrb)

            # ---- P3: output projection for this chunk's tokens ----------
            for tl in range(4):
                tt = ic * 4 + tl
                tsl = slice(tt * 128, (tt + 1) * 128)
                for nch in range(NI):
                    ps_o = ps_big.tile([128, 512], F32, name="ps", bufs=3)
                    for et in range(NE):
                        nc.tensor.matmul(
                            ps_o, ctx[et][:, tsl],
                            wo_sb[:, et, nch * 512:(nch + 1) * 512],
                            start=(et == 0), stop=(et == NE - 1))
                    ot = p_ot.tile([128, 512], F32, name="ot", bufs=4)
                    if (tl + nch) % 2 == 0:
                        nc.scalar.copy(ot, ps_o)
                    else:
                        nc.vector.tensor_copy(ot, ps_o)
                    nc.sync.dma_start(
                        out=out.ap()[tsl, nch * 512:(nch + 1) * 512], in_=ot)
            scope.__exit__(None, None, None)

        for p in (ps_sm, ps_cps, ps_big, p_rr, p_bs, p_ot, p_pt, p_x, p_res):
            p.release()

    nc.compile()
    return nc


def _classify(mask):
    """Per (ic, jt): column offset + mixed 128-col sub-blocks, from exp(mask)^T."""
    emT = np.ascontiguousarray(np.exp(mask).T)  # [key j, query i]
    pats = {}   # pattern bytes -> index
    pat_list = []
    blocks_per_ic = []
    for ic in range(NI):
        blk = []
        for jt in range(NJ):
            sub = emT[jt * 128:(jt + 1) * 128, ic * 512:(ic + 1) * 512]
            # 128-col sub-block classes
            kinds = []
            for c in range(4):
                s = sub[:, c * 128:(c + 1) * 128]
                if not s.any():
                    kinds.append(0)
                elif np.all(s == 1.0):
                    kinds.append(1)
                else:
                    kinds.append(2)
            if all(k == 0 for k in kinds):
                continue
            first = next(i for i, k in enumerate(kinds) if k != 0)
            off = first * 128
            mixed = []
            for c in range(first, 4):
                if kinds[c] != 1:
                    s = np.asarray(sub[:, c * 128:(c + 1) * 128], dtype=np.float32)
                    key = s.tobytes()
                    if key not in pats:
                        pats[key] = len(pat_list)
                        pat_list.append(s)
                    mixed.append((c, pats[key]))
            blk.append((jt, off, tuple(mixed)))
        # descending offset so the last block is full width (clean stop)
        blk.sort(key=lambda b: -b[1])
        assert blk and blk[-1][1] == 0, f"ic {ic}: no full-width block"
        blocks_per_ic.append(tuple(blk))
    em_arr = (np.concatenate(pat_list, axis=1) if pat_list
              else np.zeros((128, 128), dtype=np.float32))
    return tuple(blocks_per_ic), max(1, len(pat_list)), em_arr


def kernel(x, Wq, Wk, Wv, Wo, attn_mask):
    x = np.asarray(x, dtype=np.float32)
    Wq = np.asarray(Wq, dtype=np.float32)
    Wk = np.asarray(Wk, dtype=np.float32)
    Wv = np.asarray(Wv, dtype=np.float32)
    Wo = np.asarray(Wo, dtype=np.float32)
    mask = np.asarray(attn_mask, dtype=np.float32).reshape(T, T)

    blocks_per_ic, n_pat, em_arr = _classify(mask)
    scale = np.float32(1.0 / np.sqrt(DH))

    xT = [np.ascontiguousarray(x[b].T).astype(BF) for b in range(B)]
    em_bf = np.ascontiguousarray(em_arr).astype(BF)

    in_maps = []
    for c in range(8):
        b, g = c // 4, c % 4
        rows = slice(E * g, E * (g + 1))
        in_maps.append({
            "xt": xT[b],
            "wq": np.ascontiguousarray((Wq[rows, :] * scale).T).astype(BF),
            "wk": np.ascontiguousarray(Wk[rows, :].T).astype(BF),
            "wv": np.ascontiguousarray(Wv[rows, :].T).astype(BF),
            "wo": np.ascontiguousarray(Wo[:, rows].T).astype(BF),
            "em": em_bf.reshape(128, n_pat, 128),
            "onk": np.ones((128, 1), dtype=BF),
        })

    global _LAST_IN_MAPS, _LAST_NC
    _LAST_IN_MAPS = in_maps
    key = (blocks_per_ic, n_pat)
    if key not in _NC_CACHE:
        _NC_CACHE[key] = _build(key)
    nc = _NC_CACHE[key]
    _LAST_NC = nc
    res = run_bass_kernel_spmd(nc, in_maps, list(range(8)))
    outs = [np.asarray(r["out"], dtype=np.float32) for r in res.results]
    full = np.stack([
        outs[0] + outs[1] + outs[2] + outs[3],
        outs[4] + outs[5] + outs[6] + outs[7],
    ]).astype(np.float32)
    return full


# revision 9
# speedup vs baseline: 1.0732x; 1.0584x over previous
"""Fused multi-head attention (B=2, T=2048, D=2048, H=16) on 8 trn2 NeuronCores.

Sharding: core c handles batch b=c//4 and heads [4g, 4g+4), g=c%4 (tensor
parallel over heads x data parallel over batch). Each core computes its
4 heads' contribution to out[b] = attn(x[b]) @ Wo^T; the host sums the 4
partials per batch.

v2: single fused loop over 512-token chunks (causality: chunk ic's attention
only needs K/V from chunks <= ic), all-bf16 matmul operands (f32 PSUM),
V projected directly into [token, feature] layout (no PE transposes),
diagonal attention blocks computed at partial width with one shared
128x128 triangular mask constant.

Per chunk ic (tokens [512*ic, 512*ic+512)):
  P1  qT[m][:, chunk] = (Wq_s/sqrt(dh) @ x^T)   per m (4 feature tiles)
      kT[m][:, chunk] =  Wk_s @ x^T
      v[4ic+jl]       =  x-block^T-stationary @ Wv  -> [tok, feat]
  P2  per head h: for each surviving key block jt (descending col offset):
        S^T = kT-block^T-contract @ qT[:, off:]  (PSUM)
        pt  = exp(S^T)  (ACT, bf16)   [triangular sub-block *= tri]
        ctx^T[:, off:] += v-block^T @ pt ; l[off:] += 1^T @ pt
      ctx[h][:, chunk] = cps * broadcast(1/l)
  P3  out[t-block, :] = sum_e ctx^T[e, t-block] @ Wo -> DRAM (f32)
"""

import numpy as np
import ml_dtypes

import concourse.bass as bass
import concourse.mybir as mybir
import concourse.tile as tile
from concourse import bacc
from concourse.bass_utils import run_bass_kernel_spmd

F32 = mybir.dt.float32
F32R = mybir.dt.float32r
BF16 = mybir.dt.bfloat16
EXP = mybir.ActivationFunctionType.Exp
BF = ml_dtypes.bfloat16

B, T, D, H = 2, 2048, 2048, 16
DH = D // H          # 128
E = 512              # features per core (4 heads)
HPC = 4              # heads per core
NT = T // 128        # 16 token tiles
ND = D // 128        # 16 model-dim tiles
NE = E // 128        # 4 e-tiles per core
NI = T // 512        # 4 token chunks
NJ = NT              # 16 key tiles

_NC_CACHE = {}


def _build(blocks_key):
    # blocks_key: tuple over ic of tuple of (jt, off, mixed_tuple) where
    # mixed_tuple is ((c, pat_idx), ...) for 128-col sub-blocks needing an
    # elementwise mask multiply; n_pat = number of distinct mask patterns.
    blocks_per_ic, n_pat = blocks_key
    nc = bacc.Bacc(None, target_bir_lowering=False, debug=False)
    xt = nc.declare_dram_parameter("xt", [D, T], BF16, isOutput=False)
    wq = nc.declare_dram_parameter("wq", [D, E], BF16, isOutput=False)
    wk = nc.declare_dram_parameter("wk", [D, E], BF16, isOutput=False)
    wv = nc.declare_dram_parameter("wv", [D, E], BF16, isOutput=False)
    wo = nc.declare_dram_parameter("wo", [E, D], BF16, isOutput=False)
    em = nc.declare_dram_parameter("em", [128, n_pat, 128], BF16, isOutput=False)
    onk = nc.declare_dram_parameter("onk", [128, 1], BF16, isOutput=False)
    out = nc.declare_dram_parameter("out", [T, D], F32, isOutput=True)

    with tile.TileContext(nc) as tc:
        # ---- long-lived residents ---------------------------------------
        p_res = tc.alloc_tile_pool(name="res", bufs=1)
        qT = [p_res.tile([128, T], BF16, name=f"qT{m}") for m in range(NE)]
        kT = [p_res.tile([128, T], BF16, name=f"kT{m}") for m in range(NE)]
        ctx = [p_res.tile([128, T], BF16, name=f"ctx{m}") for m in range(NE)]
        v_sb = p_res.tile([128, NT, E], BF16)
        wq_sb = p_res.tile([128, ND, E], BF16)
        wk_sb = p_res.tile([128, ND, E], BF16)
        wv_sb = p_res.tile([128, ND, E], BF16)
        wo_sb = p_res.tile([128, NE, D], BF16)
        em_sb = p_res.tile([128, n_pat, 128], BF16)
        onk_sb = p_res.tile([128, 1], BF16)

        # ---- working pools ----------------------------------------------
        p_x = tc.alloc_tile_pool(name="px", bufs=2)
        p_pt = tc.alloc_tile_pool(name="ppt", bufs=4)
        p_ot = tc.alloc_tile_pool(name="pot", bufs=3)
        p_bs = tc.alloc_tile_pool(name="pbs", bufs=2)
        p_rr = tc.alloc_tile_pool(name="prr", bufs=2)
        ps_big = tc.alloc_tile_pool(name="psbig", bufs=4, space="PSUM")
        ps_cps = tc.alloc_tile_pool(name="pscps", bufs=2, space="PSUM")
        ps_sm = tc.alloc_tile_pool(name="pssm", bufs=1, space="PSUM")

        # DMA emission in first-use order: chunk-0 x interleaved with q/k
        # weights (P1 needs both immediately), then v weights, mask consts
        # (P2), and wo last (first P3 is ~80us in).
        xcs = {}
        xcs[0] = p_x.tile([128, ND, 512], BF16, name="xc", bufs=2)
        for dt in range(ND):
            nc.sync.dma_start(
                out=xcs[0][:, dt, :], in_=xt.ap()[dt * 128:(dt + 1) * 128, 0:512])
            nc.sync.dma_start(out=wq_sb[:, dt, :], in_=wq.ap()[dt * 128:(dt + 1) * 128, :])
            nc.sync.dma_start(out=wk_sb[:, dt, :], in_=wk.ap()[dt * 128:(dt + 1) * 128, :])
        for dt in range(ND):
            nc.sync.dma_start(out=wv_sb[:, dt, :], in_=wv.ap()[dt * 128:(dt + 1) * 128, :])
        nc.sync.dma_start(out=em_sb[:, :, :], in_=em.ap())
        nc.sync.dma_start(out=onk_sb, in_=onk.ap())
        for et in range(NE):
            nc.sync.dma_start(out=wo_sb[:, et, :], in_=wo.ap()[et * 128:(et + 1) * 128, :])

        for ic in range(NI):
            csl = slice(ic * 512, (ic + 1) * 512)
            scope = nc.named_scope(f"chunk{ic}")
            scope.__enter__()

            # ---- P1: projections for this chunk -------------------------
            # prefetch next chunk's x ahead of this chunk's output stores
            if ic + 1 < NI:
                nsl = slice((ic + 1) * 512, (ic + 2) * 512)
                xcs[ic + 1] = p_x.tile([128, ND, 512], BF16, name="xc", bufs=2)
                for dt in range(ND):
                    nc.sync.dma_start(
                        out=xcs[ic + 1][:, dt, :],
                        in_=xt.ap()[dt * 128:(dt + 1) * 128, nsl])
            xc = xcs.pop(ic)
            for m in range(NE):
                msl = slice(m * 128, (m + 1) * 128)
                psq = ps_big.tile([128, 512], F32, name="ps", bufs=4)
                psk = ps_big.tile([128, 512], F32, name="ps", bufs=4)
                if ic == 0 and m == 0:
                    # start of kernel is DMA-bound: q sweep first (needs only
                    # xc+wq), k sweep second while wk still streams in
                    for dt in range(ND):
                        nc.tensor.matmul(psq, wq_sb[:, dt, msl], xc[:, dt, :],
                                         start=dt == 0, stop=dt == ND - 1)
                    for dt in range(ND):
                        nc.tensor.matmul(psk, wk_sb[:, dt, msl], xc[:, dt, :],
                                         start=dt == 0, stop=dt == ND - 1)
                else:
                    for dt in range(ND):
                        st, sp = dt == 0, dt == ND - 1
                        nc.tensor.matmul(psq, wq_sb[:, dt, msl], xc[:, dt, :],
                                         start=st, stop=sp)
                        nc.tensor.matmul(psk, wk_sb[:, dt, msl], xc[:, dt, :],
                                         start=st, stop=sp)
                nc.scalar.copy(qT[m][:, csl], psq)
                nc.vector.tensor_copy(kT[m][:, csl], psk)
            for jl in range(4):
                jt = ic * 4 + jl
                psv = ps_big.tile([128, 512], F32, name="ps", bufs=4)
                for dt in range(ND):
                    nc.tensor.matmul(
                        psv, xc[:, dt, jl * 128:(jl + 1) * 128], wv_sb[:, dt, :],
                        start=(dt == 0), stop=(dt == ND - 1))
                nc.vector.tensor_copy(v_sb[:, jt, :], psv)

            # ---- P2: attention for this chunk ---------------------------
            blocks = blocks_per_ic[ic]
            nb = len(blocks)
            for h in range(HPC):
                hsl = slice(h * 128, (h + 1) * 128)
                cps = ps_cps.tile([128, 512], F32, name="cps", bufs=2)
                lps = ps_sm.tile([1, 512], F32, name="lps", bufs=2)
                for bi, (jt, off, mixed) in enumerate(blocks):
                    ps_s = ps_big.tile([128, 512], F32, name="ps", bufs=4)
                    nc.tensor.matmul(
                        ps_s[:, off:512], kT[h][:, jt * 128:(jt + 1) * 128],
                        qT[h][:, ic * 512 + off:(ic + 1) * 512],
                        start=True, stop=True)
                    pt = p_pt.tile([128, 512], BF16, name="pt", bufs=4)
                    nc.scalar.activation(pt[:, off:512], ps_s[:, off:512], EXP)
                    for (c, pidx) in mixed:
                        nc.vector.tensor_mul(
                            pt[:, c * 128:(c + 1) * 128],
                            pt[:, c * 128:(c + 1) * 128],
                            em_sb[:, pidx, :])
                    st, sp = bi == 0, bi == nb - 1
                    nc.tensor.matmul(cps[:, off:512], v_sb[:, jt, hsl],
                                     pt[:, off:512], start=st, stop=sp)
                    nc.tensor.matmul(lps[:, off:512], onk_sb, pt[:, off:512],
                                     start=st, stop=sp)
                rr = p_rr.tile([1, 512], F32, name="rr", bufs=2)
                nc.vector.reciprocal_approx_fast(out=rr, in_=lps)
                rrb = p_bs.tile([128, 512], F32, name="rrb", bufs=2)
                nc.gpsimd.partition_broadcast(rrb, rr)
                nc.vector.tensor_mul(ctx[h][:, csl], cps, rrb)

            # ---- P3: output projection for this chunk's tokens ----------
            for tl in range(4):
                tt = ic * 4 + tl
                tsl = slice(tt * 128, (tt + 1) * 128)
                for nch in range(NI):
                    ps_o = ps_big.tile([128, 512], F32, name="ps", bufs=4)
                    for et in range(NE):
                        nc.tensor.matmul(
                            ps_o, ctx[et][:, tsl],
                            wo_sb[:, et, nch * 512:(nch + 1) * 512],
                            start=(et == 0), stop=(et == NE - 1))
                    ot = p_ot.tile([128, 512], F32, name="ot", bufs=4)
                    if (tl + nch) % 2 == 0:
                        nc.scalar.copy(ot, ps_o)
                    else:
                        nc.vector.tensor_copy(ot, ps_o)
                    nc.sync.dma_start(
                        out=out.ap()[tsl, nch * 512:(nch + 1) * 512], in_=ot)
            scope.__exit__(None, None, None)

        for p in (ps_sm, ps_cps, ps_big, p_rr, p_bs, p_ot, p_pt, p_x, p_res):
            p.release()

    nc.compile()
    return nc


def _classify(mask):
    """Per (ic, jt): column offset + mixed 128-col sub-blocks, from exp(mask)^T."""
    emT = np.ascontiguousarray(np.exp(mask).T)  # [key j, query i]
    pats = {}   # pattern bytes -> index
    pat_list = []
    blocks_per_ic = []
    for ic in range(NI):
        blk = []
        for jt in range(NJ):
            sub = emT[jt * 128:(jt + 1) * 128, ic * 512:(ic + 1) * 512]
            # 128-col sub-block classes
            kinds = []
            for c in range(4):
                s = sub[:, c * 128:(c + 1) * 128]
                if not s.any():
                    kinds.append(0)
                elif np.all(s == 1.0):
                    kinds.append(1)
                else:
                    kinds.append(2)
            if all(k == 0 for k in kinds):
                continue
            first = next(i for i, k in enumerate(kinds) if k != 0)
            off = first * 128
            mixed = []
            for c in range(first, 4):
                if kinds[c] != 1:
                    s = np.asarray(sub[:, c * 128:(c + 1) * 128], dtype=np.float32)
                    key = s.tobytes()
                    if key not in pats:
                        pats[key] = len(pat_list)
                        pat_list.append(s)
                    mixed.append((c, pats[key]))
            blk.append((jt, off, tuple(mixed)))
        # descending offset so the last block is full width (clean stop)
        blk.sort(key=lambda b: -b[1])
        assert blk and blk[-1][1] == 0, f"ic {ic}: no full-width block"
        blocks_per_ic.append(tuple(blk))
    em_arr = (np.concatenate(pat_list, axis=1) if pat_list
              else np.zeros((128, 128), dtype=np.float32))
    return tuple(blocks_per_ic), max(1, len(pat_list)), em_arr


def kernel(x, Wq, Wk, Wv, Wo, attn_mask):
    x = np.asarray(x, dtype=np.float32)
    Wq = np.asarray(Wq, dtype=np.float32)
    Wk = np.asarray(Wk, dtype=np.float32)
    Wv = np.asarray(Wv, dtype=np.float32)
    Wo = np.asarray(Wo, dtype=np.float32)
    mask = np.asarray(attn_mask, dtype=np.float32).reshape(T, T)

    blocks_per_ic, n_pat, em_arr = _classify(mask)
    scale = np.float32(1.0 / np.sqrt(DH))

    xT = [np.ascontiguousarray(x[b].T).astype(BF) for b in range(B)]
    em_bf = np.ascontiguousarray(em_arr).astype(BF)

    in_maps = []
    for c in range(8):
        b, g = c // 4, c % 4
        rows = slice(E * g, E * (g + 1))
        in_maps.append({
            "xt": xT[b],
            "wq": np.ascontiguousarray((Wq[rows, :] * scale).T).astype(BF),
            "wk": np.ascontiguousarray(Wk[rows, :].T).astype(BF),
            "wv": np.ascontiguousarray(Wv[rows, :].T).astype(BF),
            "wo": np.ascontiguousarray(Wo[:, rows].T).astype(BF),
            "em": em_bf.reshape(128, n_pat, 128),
            "onk": np.ones((128, 1), dtype=BF),
        })

    global _LAST_IN_MAPS, _LAST_NC
    _LAST_IN_MAPS = in_maps
    key = (blocks_per_ic, n_pat)
    if key not in _NC_CACHE:
        _NC_CACHE[key] = _build(key)
    nc = _NC_CACHE[key]
    _LAST_NC = nc
    res = run_bass_kernel_spmd(nc, in_maps, list(range(8)))
    outs = [np.asarray(r["out"], dtype=np.float32) for r in res.results]
    full = np.stack([
        outs[0] + outs[1] + outs[2] + outs[3],
        outs[4] + outs[5] + outs[6] + outs[7],
    ]).astype(np.float32)
    return full
